# revision 24
# baseline (speedup 1.0000x reference)
"""DGCN layer on 8 TRN2 NeuronCores (Bass/Tile).

Strategy (row-parallel SpMM, hint-aligned):
  - Rows (users/items) sharded across 8 cores: core c owns rows
    [c*12500, (c+1)*12500) of both U and V spaces.
  - Dense supports s_k = x @ W_k computed row-parallel (bf16), AllGather'd
    to every core's DRAM.
  - Each SpMM: per output row-tile (128 rows), edges are gathered from the
    support table via dma_gather (256B bf16 rows), then segment-summed on
    the PE: for each 128-edge chunk, a scatter matrix S.T [128e, 128r]
    (S.T[e, r] = val[e] * (rowlocal[e] == r)) is built in ONE DVE
    tensor_scalar (dual-scalar: is_equal then mult) and matmul-accumulated
    into a PSUM tile [128f, 128r] (out.T layout).
  - Epilogues fuse PReLU(+bias) on ACT and the next dense GEMM on PE
    (the [f, r] layout is exactly the lhsT the next GEMM needs).
  - Union linears: two accumulating matmuls + a K=1 matmul adding the bias
    row, then ReLU on DVE.

SPMD: one program for all 8 cores. All instruction-stream shapes (chunk
counts per (tile, block) bucket) are uniformized across cores by padding
each bucket to a global cap with (idx=0, val=0) edges.
"""

import numpy as np
import ml_dtypes
from contextlib import ExitStack

BF16 = ml_dtypes.bfloat16

NC = 8
F = 128
ST_T = 4                   # row-tiles per gather super-tile
ALPHA = 0.2

# problem dims — overridable via _set_dims for scaled-down testing
N = 100000                 # U == V
RPC = N // NC              # rows per core
NBLK = 4                   # int16 index blocks over the support table
BLK = N // NBLK            # rows per block
NT = (RPC + 127) // 128    # row-tiles per core
TAIL = RPC - 128 * (NT - 1)

LAST_EXEC_NS = None


def _set_dims(n):
    global N, RPC, BLK, NT, TAIL
    N = n
    RPC = N // NC
    BLK = N // NBLK
    NT = (RPC + 127) // 128
    TAIL = RPC - 128 * (NT - 1)


# ---------------------------------------------------------------- host prep

class AdjLayout:
    """Static (core-invariant) instruction-stream layout for one adjacency."""

    def __init__(self, c_full, c_tail):
        self.c_full, self.c_tail = c_full, c_tail
        self.C = [c_full] * (NT - 1) + [c_tail]
        self.sts = [list(range(i, min(i + ST_T, NT)))
                    for i in range(0, NT, ST_T)]
        self.pstart = np.zeros((NT, NBLK), np.int64)
        self.calls = []   # (st_i, b, edge_start, num_idxs)
        off = 0
        for si, stt in enumerate(self.sts):
            for b in range(NBLK):
                cs = off
                for t in stt:
                    self.pstart[t, b] = off
                    off += self.C[t] * 128
                self.calls.append((si, b, cs, off - cs))
        self.ntot = off
        self.nch = off // 128

    def chunk_col(self, t, b, k):
        return int(self.pstart[t, b]) // 128 + k

    def rank(self, st_edge_start, t, b, k):
        return (int(self.pstart[t, b]) - st_edge_start) // 128 + k


def _prep_adj(rows, cols, vals):
    """rows sorted. Returns (layout, per-core dict of idx/rr/vv arrays)."""
    rows = np.asarray(rows, np.int64)
    cols = np.asarray(cols, np.int64)
    vals = np.asarray(vals, np.float32)
    bounds = np.searchsorted(rows, np.arange(NC + 1) * RPC, side="left")

    # pass 1: bucket counts -> global caps
    per_core = []
    cmax_full = 0
    cmax_tail = 0
    for c in range(NC):
        sl = slice(bounds[c], bounds[c + 1])
        r_loc = rows[sl] - c * RPC
        t_id = r_loc >> 7
        b_id = cols[sl] // BLK
        key = (t_id * NBLK + b_id).astype(np.int64)
        cnt = np.bincount(key, minlength=NT * NBLK).reshape(NT, NBLK)
        per_core.append((sl, r_loc, t_id, b_id, key, cnt))
        cmax_full = max(cmax_full, int(cnt[:NT - 1].max()))
        cmax_tail = max(cmax_tail, int(cnt[NT - 1].max()))
    L = AdjLayout((cmax_full + 127) // 128, max(1, (cmax_tail + 127) // 128))

    # pass 2: fill padded arrays
    cores = []
    for c in range(NC):
        sl, r_loc, t_id, b_id, key, cnt = per_core[c]
        n = r_loc.shape[0]
        order = np.argsort(key, kind="stable")
        ks = key[order]
        starts = np.zeros(NT * NBLK + 1, np.int64)
        np.cumsum(cnt.reshape(-1), out=starts[1:])
        within = np.arange(n, dtype=np.int64) - starts[ks]
        dest = L.pstart.reshape(-1)[ks] + within

        idx_flat = np.zeros(L.ntot, np.int16)
        rr_flat = np.zeros(L.ntot, np.float32)
        vv_flat = np.zeros(L.ntot, np.float32)
        idx_flat[dest] = (cols[sl][order] - b_id[order] * BLK).astype(np.int16)
        rr_flat[dest] = (r_loc[order] - (t_id[order] << 7)).astype(np.float32)
        vv_flat[dest] = vals[sl][order]

        idx16 = np.ascontiguousarray(
            np.tile(idx_flat.reshape(-1, 16).T, (8, 1)))      # [128, ntot/16]
        rr = np.ascontiguousarray(rr_flat.reshape(-1, 128).T)  # [128, nch]
        vv = np.ascontiguousarray(vv_flat.reshape(-1, 128).T)
        cores.append(dict(idx=idx16, rr=rr, vv=vv))
    return L, cores


# ------------------------------------------------------------- bass program

def _build(LVU, LUV, qmap=None):
    """qmap: list of queue_num per gather issue-ordinal (fixpoint-assigned so
    queue matches the tile sem-assigner's DMASW lane rotation, which follows
    the SCHEDULED order of Pool DMA instructions)."""
    import concourse.bacc as bacc
    import concourse.mybir as mybir
    from concourse.tile import TileContext

    dt = mybir.dt
    AOT = mybir.AluOpType
    ACTF = mybir.ActivationFunctionType

    nc = bacc.Bacc("TRN2", num_devices=NC, num_swdge_queues=4)

    def din(name, shape, dty):
        return nc.dram_tensor(name, shape, dty, kind="ExternalInput")

    feaT_u = din("feaT_u", [F, RPC], dt.bfloat16)
    feaT_v = din("feaT_v", [F, RPC], dt.bfloat16)
    feaT_u_full = din("feaT_u_full", [F, N], dt.bfloat16)
    feaT_v_full = din("feaT_v_full", [F, N], dt.bfloat16)
    wts = {k: din(k, [F, F], dt.bfloat16)
           for k in ["W1", "W2", "W3", "W4", "Wut", "Wub", "Wit", "Wib"]}
    biases = {k: din(k, [F, 1], dt.float32) for k in ["b1", "b2", "b3", "b4"]}
    bu_row = din("bu_row", [1, F], dt.bfloat16)
    bi_row = din("bi_row", [1, F], dt.bfloat16)
    iota_in = din("iota", [128, 128], dt.bfloat16)
    ones_in = din("ones", [1, 128], dt.bfloat16)

    meta = {}
    for tag, L in [("vu", LVU), ("uv", LUV)]:
        meta[tag] = dict(
            idx=din(f"idx_{tag}", [128, L.ntot // 16], dt.int16),
            rr=din(f"rr_{tag}", [128, L.nch], dt.float32),
            vv=din(f"vv_{tag}", [128, L.nch], dt.float32),
            L=L,
        )

    out_user = nc.dram_tensor("user_out", [RPC, F], dt.float32,
                              kind="ExternalOutput")
    out_item = nc.dram_tensor("item_out", [RPC, F], dt.float32,
                              kind="ExternalOutput")

    gather_insts = []

    shards = {k: nc.dram_tensor(f"{k}_shard", [RPC, F], dt.bfloat16,
                                kind="Internal") for k in ["s3", "s4"]}
    fulls = {k: nc.dram_tensor(f"{k}_full", [N, F], dt.bfloat16,
                               kind="Internal", addr_space="Shared")
             for k in ["s1", "s2", "s3", "s4"]}

    with TileContext(nc) as tc, ExitStack() as ctx:
        consts = ctx.enter_context(tc.tile_pool(name="consts", bufs=1))
        metap = ctx.enter_context(tc.tile_pool(name="meta", bufs=1))
        idxp = ctx.enter_context(tc.tile_pool(name="idx", bufs=3))
        gp = ctx.enter_context(tc.tile_pool(name="gather", bufs=2))
        sp = ctx.enter_context(tc.tile_pool(name="sT", bufs=4))
        hp = ctx.enter_context(tc.tile_pool(name="hidden", bufs=4))
        ob = ctx.enter_context(tc.tile_pool(name="outsb", bufs=3))
        psA = ctx.enter_context(tc.tile_pool(name="psA", bufs=6, space="PSUM"))
        ps2 = ctx.enter_context(tc.tile_pool(name="ps2", bufs=2, space="PSUM"))

        def cload(handle, shape, dty, tag):
            t = consts.tile(shape, dty, tag=tag)
            nc.sync.dma_start(t[:], handle[:])
            return t

        ufeaT = cload(feaT_u, [F, RPC], dt.bfloat16, "c_ufeaT")
        vfeaT = cload(feaT_v, [F, RPC], dt.bfloat16, "c_vfeaT")
        wt = {k: cload(v, [F, F], dt.bfloat16, f"c_{k}")
              for k, v in wts.items()}
        bt = {k: cload(v, [F, 1], dt.float32, f"c_{k}")
              for k, v in biases.items()}
        bu_t = cload(bu_row, [1, F], dt.bfloat16, "c_bu")
        bi_t = cload(bi_row, [1, F], dt.bfloat16, "c_bi")
        iota = cload(iota_in, [128, 128], dt.bfloat16, "c_iota")
        ones = cload(ones_in, [1, 128], dt.bfloat16, "c_ones")

        def rows_of(t):
            return 128 if t < NT - 1 else TAIL

        # ---------------- stage 0: full s1 = ufea@W1, s2 = vfea@W2, computed
        # redundantly on every core from the replicated feature tables (kills
        # the s1/s2 AllGathers; PE is idle here anyway).
        NFT = (N + 127) // 128
        CH = 8                      # support-table tiles per feature DMA
        fq = ctx.enter_context(tc.tile_pool(name="feach", bufs=3))
        for feat_full, w, skey in ((feaT_u_full, "W1", "s1"),
                                   (feaT_v_full, "W2", "s2")):
            for c0 in range(0, NFT, CH):
                ctiles = min(CH, NFT - c0)
                cols = min(N - c0 * 128, ctiles * 128)
                fc = fq.tile([128, CH * 128], dt.bfloat16, tag="fc")
                nc.sync.dma_start(fc[:, :cols],
                                  feat_full[:, c0 * 128:c0 * 128 + cols])
                for k in range(ctiles):
                    t = c0 + k
                    R = min(128, N - t * 128)
                    ps = ps2.tile([128, F], dt.float32, tag="ps2")
                    nc.tensor.matmul(ps[:R, :], fc[:, k * 128:k * 128 + R],
                                     wt[w][:], start=True, stop=True)
                    sb = hp.tile([128, F], dt.bfloat16, tag="h")
                    nc.vector.tensor_copy(sb[:R, :], ps[:R, :])
                    nc.sync.dma_start(fulls[skey][t * 128:t * 128 + R, :],
                                      sb[:R, :])

        def allgather(k):
            nc.gpsimd.collective_compute(
                "AllGather", AOT.bypass,
                replica_groups=[list(range(NC))],
                ins=[shards[k][:]], outs=[fulls[k][:]],
            )

        # ---------------- generic spmm stage
        import os as _os2
        ablate = _os2.environ.get("KB_ABLATE", "")

        def spmm(adj, s_key, epi, inject=None):
            m = meta[adj]
            L = m["L"]
            rr = metap.tile([128, L.nch], dt.float32, tag="rr")
            nc.sync.dma_start(rr[:], m["rr"][:])
            vv = metap.tile([128, L.nch], dt.float32, tag="vv")
            nc.sync.dma_start(vv[:], m["vv"][:])
            sfull = fulls[s_key]

            sT0 = None
            if ablate == "dve":
                sT0 = sp.tile([128, 128], dt.bfloat16, tag="sT")
                nc.vector.tensor_scalar(
                    sT0[:], iota[:], rr[:, 0:1], vv[:, 0:1],
                    AOT.is_equal, AOT.mult)
            it0 = None

            for si, stt in enumerate(L.sts):
                if si == 1 and inject is not None:
                    inject()
                # one idx DMA per super-tile (the 4 block calls are
                # contiguous in the idx table)
                st_cs = L.calls[si * NBLK][2]
                st_n = sum(L.calls[si * NBLK + b][3] for b in range(NBLK))
                if ablate == "idx":
                    if it0 is None:
                        it0 = idxp.tile([128, st_n // 16], dt.int16,
                                        tag="idx")
                        nc.sync.dma_start(
                            it0[:],
                            m["idx"][:, st_cs // 16: (st_cs + st_n) // 16])
                    it = it0
                else:
                    it = idxp.tile([128, st_n // 16], dt.int16, tag="idx")
                    nc.sync.dma_start(
                        it[:], m["idx"][:, st_cs // 16: (st_cs + st_n) // 16])
                gbufs = []
                cstarts = []
                for b in range(NBLK):
                    _, _, cs, nidx = L.calls[si * NBLK + b]
                    g = gp.tile([128, nidx // 128, F], dt.bfloat16,
                                tag=f"g{b}")
                    ordinal = len(gather_insts)
                    qn = qmap[ordinal] if qmap else ordinal % 4
                    gn = 128 if ablate == "gather" else nidx
                    co = (cs - st_cs) // 16
                    inst = nc.gpsimd.dma_gather(
                        g[:, :gn // 128, :], sfull[b * BLK:(b + 1) * BLK, :],
                        it[:, co:co + nidx // 16], gn, gn, F,
                        single_packet=False, queue_num=qn)
                    gather_insts.append(inst)
                    gbufs.append(g)
                    cstarts.append(cs)
                # block-major chunk consumption: finish every tile's block-b
                # chunks before touching block b+1, so gbuf[b] frees as early
                # as possible and the next super-tile's gathers pipeline in.
                psTs = {}
                for t in stt:
                    psTs[t] = psA.tile([128, 128], dt.float32, tag="psA",
                                       name=f"psT_{t}")
                for b in range(NBLK):
                    for t in stt:
                        for k in range(L.C[t]):
                            j = L.chunk_col(t, b, k)
                            rk = L.rank(cstarts[b], t, b, k)
                            first = (b == 0 and k == 0)
                            last = (b == NBLK - 1 and k == L.C[t] - 1)
                            if ablate == "pe" and not first:
                                continue
                            if ablate == "dve":
                                sT = sT0
                            else:
                                sT = sp.tile([128, 128], dt.bfloat16, tag="sT")
                                nc.vector.tensor_scalar(
                                    sT[:], iota[:], rr[:, j:j + 1],
                                    vv[:, j:j + 1], AOT.is_equal, AOT.mult)
                            rk2 = 0 if ablate == "gather" else rk
                            nc.tensor.matmul(
                                psTs[t][:], gbufs[b][:, rk2, :], sT[:],
                                start=first,
                                stop=(last or ablate == "pe"),
                                skip_group_check=True)
                        if b == NBLK - 1:
                            epi(t, psTs[t])

        # epilogue: hidden-layer -> next support shard
        def epi_hidden(bias_key, w_key, sh_key):
            def epi(t, psT):
                R = rows_of(t)
                hT = hp.tile([128, 128], dt.bfloat16, tag="h")
                nc.scalar.activation(hT[:], psT[:], ACTF.Prelu,
                                     bias=bt[bias_key][:, 0:1], scale=1.0,
                                     alpha=ALPHA)
                ps = ps2.tile([128, F], dt.float32, tag="ps2")
                nc.tensor.matmul(ps[:R, :], hT[:, :R], wt[w_key][:],
                                 start=True, stop=True)
                sb = hp.tile([128, F], dt.bfloat16, tag="h")
                nc.vector.tensor_copy(sb[:R, :], ps[:R, :])
                nc.sync.dma_start(shards[sh_key][t * 128:t * 128 + R, :],
                                  sb[:R, :])
            return epi

        # epilogue: final layer -> union linear -> output
        def epi_union(bias_key, wt_key, wb_key, feat, brow, out_t):
            def epi(t, psT):
                R = rows_of(t)
                hT = hp.tile([128, 128], dt.bfloat16, tag="h")
                nc.scalar.activation(hT[:], psT[:], ACTF.Prelu,
                                     bias=bt[bias_key][:, 0:1], scale=1.0,
                                     alpha=ALPHA)
                ps = ps2.tile([128, F], dt.float32, tag="ps2")
                nc.tensor.matmul(ps[:R, :], hT[:, :R], wt[wt_key][:],
                                 start=True, stop=False, skip_group_check=True)
                nc.tensor.matmul(ps[:R, :], feat[:, t * 128:t * 128 + R],
                                 wt[wb_key][:], start=False, stop=False,
                                 skip_group_check=True)
                nc.tensor.matmul(ps[:R, :], ones[0:1, :R], brow[0:1, :],
                                 start=False, stop=True, skip_group_check=True)
                osb = ob.tile([128, F], dt.float32, tag="o")
                nc.vector.tensor_scalar_max(osb[:R, :], ps[:R, :], 0.0)
                nc.sync.dma_start(out_t[t * 128:t * 128 + R, :], osb[:R, :])
            return epi

        import os
        lvl = int(os.environ.get("KB_STAGES", "6"))
        import os as _os
        agmode = int(_os.environ.get("KB_AGMODE", "0"))
        if lvl >= 2:
            spmm("vu", "s1", epi_hidden("b1", "W3", "s3"))   # A
        if lvl >= 3:
            # AG(s3) injected after B's first gather super-tile so its
            # sem-wait on A's epilogue stores doesn't stall the Pool queue,
            # and the transfer overlaps B's remaining gathers.
            spmm("uv", "s2", epi_hidden("b2", "W4", "s4"),
                 inject=(lambda: allgather("s3"))
                 if (lvl >= 4 and agmode == 0) else None)  # B
        if lvl >= 4 and agmode == 1:
            allgather("s3")
        if lvl >= 5:
            spmm("uv", "s3",
                 epi_union("b3", "Wut", "Wub", ufeaT, bu_t, out_user),
                 inject=(lambda: allgather("s4")) if agmode == 0 else None)
        if lvl >= 5 and agmode == 1:
            allgather("s4")
        if lvl >= 6:
            spmm("vu", "s4",
                 epi_union("b4", "Wit", "Wib", vfeaT, bi_t, out_item))

    nc.compile()
    return nc, gather_insts


def _scheduled_queue_targets(nc, gather_insts):
    """Desired queue per gather issue-ordinal: the tile sem assigner rotates
    Pool DMA instructions over 8 DMASW lanes in SCHEDULED order; a lane must
    only ever see one SWDGE queue, so queue must equal (sched position % 4)
    (lane = pos % 8, and pos % 4 keeps each lane single-queue)."""
    import concourse.mybir as mybir
    from concourse.tile_scheduler import DMAInst
    from concourse import bass_isa

    name_to_ord = {inst.ins.name: i for i, inst in enumerate(gather_insts)}
    desired = [0] * len(gather_insts)
    pure = True
    p = 0
    for blk in nc.m.functions[0].blocks:
        for inst in blk.instructions:
            if inst.engine != mybir.EngineType.Pool:
                continue
            if isinstance(inst, DMAInst) and not isinstance(
                    inst, bass_isa.UserSyncedRemoteDMADescs):
                lane_q = p % 4
                o = name_to_ord.get(inst.name)
                if o is not None:
                    desired[o] = lane_q
                    if inst.queue_num != lane_q:
                        pure = False
                elif lane_q != 0:
                    pure = False   # stray queue-0 Pool DMA on a nonzero lane
                p += 1
    return desired, pure


def _build_fixpoint(LVU, LUV, max_passes=3):
    qmap = None
    for _ in range(max_passes):
        nc, ginsts = _build(LVU, LUV, qmap)
        desired, pure = _scheduled_queue_targets(nc, ginsts)
        if pure:
            return nc
        qmap = desired
    nc, _ = _build(LVU, LUV, [0] * len(qmap))   # safe single-queue fallback
    return nc


# ------------------------------------------------------------------ driver

def _run_and_time(nc, in_maps, iters=2):
    """Mirror bass2jax.run_bass_via_pjrt's multi-core path, minus donation,
    so the same compiled executable can be re-invoked for wall-clock timing
    with device-resident inputs."""
    import os
    import time
    import jax
    from jax.sharding import Mesh, PartitionSpec, NamedSharding
    from jax.experimental.shard_map import shard_map
    from concourse import bass2jax
    import concourse.mybir as mybir

    bass2jax.install_neuronx_cc_hook()
    part_name = nc.partition_id_tensor.name if nc.partition_id_tensor else None
    in_names, out_names, out_avals, zero_outs = [], [], [], []
    for alloc in nc.m.functions[0].allocations:
        if not isinstance(alloc, mybir.MemoryLocationSet):
            continue
        name = alloc.memorylocations[0].name
        if alloc.kind == "ExternalInput":
            if name != part_name:
                in_names.append(name)
        elif alloc.kind == "ExternalOutput":
            out_names.append(name)
            shape = tuple(alloc.tensor_shape)
            dty = mybir.dt.np(alloc.dtype)
            out_avals.append(jax.core.ShapedArray(shape, dty))
            zero_outs.append(np.zeros(shape, dty))
    n_params = len(in_names)
    all_in = list(in_names) + list(out_names)
    if part_name:
        all_in.append(part_name)

    def _body(*args):
        operands = list(args)
        if part_name:
            operands.append(bass2jax.partition_id_tensor())
        outs = bass2jax._bass_exec_p.bind(
            *operands, out_avals=tuple(out_avals), in_names=tuple(all_in),
            out_names=tuple(out_names), lowering_input_output_aliases=(),
            sim_require_finite=True, sim_require_nnan=True, nc=nc)
        return tuple(outs)

    devices = jax.devices()[:NC]
    mesh = Mesh(np.asarray(devices), ("core",))
    nio = n_params + len(out_names)
    sharded = jax.jit(
        shard_map(_body, mesh=mesh, in_specs=(PartitionSpec("core"),) * nio,
                  out_specs=(PartitionSpec("core"),) * len(out_names),
                  check_rep=False),
        keep_unused=True)

    sh = NamedSharding(mesh, PartitionSpec("core"))
    dev_in = [jax.device_put(
        np.concatenate([np.asarray(m[name]) for m in in_maps], 0), sh)
        for name in in_names]
    dev_zero = [jax.device_put(
        np.zeros((NC * z.shape[0], *z.shape[1:]), z.dtype), sh)
        for z in zero_outs]

    out = sharded(*dev_in, *dev_zero)
    jax.block_until_ready(out)
    results = [
        {name: np.asarray(out[i]).reshape(NC, *out_avals[i].shape)[c]
         for i, name in enumerate(out_names)}
        for c in range(NC)]

    # Throughput timing: the axon tunnel adds ~67 ms of round-trip latency
    # per synchronized call, which is not HW execution time. Launch a batch
    # of back-to-back executions (device serializes them) and divide.
    npipe = int(os.environ.get("KERNEL_PIPE_N", "64"))
    best = None
    for _ in range(iters):
        jax.block_until_ready(sharded(*dev_in, *dev_zero))
        t0 = time.perf_counter()
        outs = [sharded(*dev_in, *dev_zero) for _ in range(npipe)]
        jax.block_until_ready(outs)
        dtns = (time.perf_counter() - t0) * 1e9 / npipe
        best = dtns if best is None else min(best, dtns)
    return results, best


def kernel(**inputs):
    global LAST_EXEC_NS

    ufea = np.asarray(inputs["ufea"], np.float32)
    vfea = np.asarray(inputs["vfea"], np.float32)

    LVU, vu_cores = _prep_adj(inputs["vu_rows"], inputs["vu_cols"],
                              inputs["vu_vals"])
    LUV, uv_cores = _prep_adj(inputs["uv_rows"], inputs["uv_cols"],
                              inputs["uv_vals"])

    nc = _build_fixpoint(LVU, LUV)

    W = {k: np.asarray(inputs[k], np.float32) for k in
         ["W1", "b1", "W2", "b2", "W3", "b3", "W4", "b4",
          "Wu", "bu", "Wi", "bi"]}
    common = dict(
        W1=W["W1"].astype(BF16), W2=W["W2"].astype(BF16),
        W3=W["W3"].astype(BF16), W4=W["W4"].astype(BF16),
        Wut=np.ascontiguousarray(W["Wu"][:F]).astype(BF16),
        Wub=np.ascontiguousarray(W["Wu"][F:]).astype(BF16),
        Wit=np.ascontiguousarray(W["Wi"][:F]).astype(BF16),
        Wib=np.ascontiguousarray(W["Wi"][F:]).astype(BF16),
        b1=W["b1"].reshape(F, 1), b2=W["b2"].reshape(F, 1),
        b3=W["b3"].reshape(F, 1), b4=W["b4"].reshape(F, 1),
        bu_row=W["bu"].reshape(1, F).astype(BF16),
        bi_row=W["bi"].reshape(1, F).astype(BF16),
        iota=np.tile(np.arange(128).astype(BF16)[None, :], (128, 1)),
        ones=np.ones((1, 128), BF16),
    )

    ufeaT_full = np.ascontiguousarray(ufea.T).astype(BF16)
    vfeaT_full = np.ascontiguousarray(vfea.T).astype(BF16)
    in_maps = []
    for c in range(NC):
        m = dict(common)
        m["feaT_u"] = np.ascontiguousarray(ufeaT_full[:, c * RPC:(c + 1) * RPC])
        m["feaT_v"] = np.ascontiguousarray(vfeaT_full[:, c * RPC:(c + 1) * RPC])
        m["feaT_u_full"] = ufeaT_full
        m["feaT_v_full"] = vfeaT_full
        for tag, cores in (("vu", vu_cores), ("uv", uv_cores)):
            m[f"idx_{tag}"] = cores[c]["idx"]
            m[f"rr_{tag}"] = cores[c]["rr"]
            m[f"vv_{tag}"] = cores[c]["vv"]
        in_maps.append(m)

    results, wall_ns = _run_and_time(
        nc, in_maps,
        iters=int(__import__("os").environ.get("KERNEL_BENCH_ITERS", "3")))
    LAST_EXEC_NS = int(wall_ns)

    user = np.concatenate([results[c]["user_out"] for c in range(NC)], 0)
    item = np.concatenate([results[c]["item_out"] for c in range(NC)], 0)
    return (user, item)

